# revision 9
# baseline (speedup 1.0000x reference)
"""Bidirectional 2-layer LSTM decoder on 8 Trainium2 NeuronCores.

Structure (hardcoded for B=64, T=256, I=H=1024):
  - reference semantics: 4 LSTM cells per step (layer0 fwd/bwd on x /
    feature-reversed x, layer1 fwd/bwd on layer0 outputs), all scanning
    forward in time.
  - sharding: core = (cell, batch-half) -> 8 cores, 32 batch rows each.
    Zero communication inside a launch; layer-1 cores consume layer-0 h
    sequences handed off host-side between chunked launches (software
    pipeline with chunk size C steps).
  - per-core kernel (one chunk): dense GEMM for the x-part of the gates
    (x @ W_ih.T + bias, full PE utilization), then C recurrent steps
    (h @ W_hh.T in two concurrent PE column-group chains, gate fold,
    sigmoid/tanh on ScalarE, gate math on VectorE, h transpose on PE).
  - all matmuls bf16 (fp32 is 4x slower on the PE); cell state c in fp32.
"""

import os
import sys

sys.path.insert(0, "/opt/trn_rl_repo")

import numpy as np
import ml_dtypes

import concourse.bass as bass
import concourse.mybir as mybir
import concourse.tile as tile
from concourse import bacc
from concourse.bass_utils import run_bass_kernel_spmd
from concourse.masks import make_identity


def _install_ntff_shim():
    """Provide antenv.axon_hooks (missing in this image) so that
    run_bass_kernel_spmd(trace=True) can capture NTFF profiles."""
    import types

    try:
        import antenv.axon_hooks  # noqa: F401
        return
    except ImportError:
        pass
    try:
        import antenv
        from trn_agent_boot.trn_boot import _ntff_profile_via_ctypes

        mod = types.ModuleType("antenv.axon_hooks")
        _h = [None]
        mod.set_axon_ntff_profile_hook = lambda h: _h.__setitem__(0, h)
        mod.get_axon_ntff_profile_hook = lambda: _h[0]
        sys.modules["antenv.axon_hooks"] = mod
        antenv.axon_hooks = mod
        hook = _ntff_profile_via_ctypes("/opt/axon/libaxon_pjrt.so")
        mod.set_axon_ntff_profile_hook(hook)

        import concourse.bass_utils as _bu
        _bu.upload_artifacts = lambda tmpdir: tmpdir  # no bucket in container
    except Exception:
        pass


_install_ntff_shim()

BF16 = mybir.dt.bfloat16
F32 = mybir.dt.float32
NPBF = ml_dtypes.bfloat16

B, T, I, H = 64, 256, 1024, 1024
G = 4 * H            # gate width
NK = H // 128        # K tiles (8)
NCH = G // 512       # gate chunks of 512 (8)
BH = B // 2          # batch half = 32
C = 64               # scan steps per launch
R = C * BH           # dense rows per chunk (2048)

_CACHE = {}


def _build_nc(C=C, R=None, debug=False):
    R = C * BH if R is None else R
    nc = bacc.Bacc("TRN2", target_bir_lowering=False, debug=debug)

    xin_d = nc.dram_tensor("xin", (R, I), BF16, kind="ExternalInput")
    wih_d = nc.dram_tensor("wihT", (128, NK, G), BF16, kind="ExternalInput")
    whh_d = nc.dram_tensor("whhT", (128, NK, G), BF16, kind="ExternalInput")
    brow_d = nc.dram_tensor("brow", (1, G), BF16, kind="ExternalInput")
    hT0_d = nc.dram_tensor("hT0", (128, NK, BH), BF16, kind="ExternalInput")
    c0_d = nc.dram_tensor("c0", (BH, H), F32, kind="ExternalInput")

    hseq_d = nc.dram_tensor("hseq", (C, BH, H), BF16, kind="ExternalOutput")
    hTn_d = nc.dram_tensor("hTn", (128, NK, BH), BF16, kind="ExternalOutput")
    cn_d = nc.dram_tensor("cn", (BH, H), F32, kind="ExternalOutput")

    Xg_d = nc.dram_tensor("Xg", (R, G), BF16, kind="Internal")

    with tile.TileContext(nc) as tc:
        with (
            tc.tile_pool(name="const", bufs=1) as cst,
            tc.tile_pool(name="wpool", bufs=2) as wpool,
            tc.tile_pool(name="xtp", bufs=2) as xtp,
            tc.tile_pool(name="gp", bufs=2) as gp,
            tc.tile_pool(name="ep", bufs=1) as ep,
            tc.tile_pool(name="cp", bufs=2) as cp,
            tc.tile_pool(name="hp", bufs=2) as hp,
            tc.tile_pool(name="hTp", bufs=2) as hTp,
            tc.tile_pool(name="xev", bufs=4) as xev,
            tc.tile_pool(name="psd", bufs=2, space="PSUM") as psd,
            tc.tile_pool(name="psg", bufs=4, space="PSUM") as psg,
            tc.tile_pool(name="pst", bufs=2, space="PSUM") as pst,
        ):
            # ---- constants / weights ----
            whh_sb = cst.tile([128, NK, G], BF16)
            brow_sb = cst.tile([1, G], BF16)
            i32 = cst.tile([32, 32], BF16)
            ones = cst.tile([1, 128], BF16)
            xT_sb = cst.tile([128, NK, R], BF16)

            nc.sync.dma_start(whh_sb[:], whh_d[:])
            nc.sync.dma_start(brow_sb[:], brow_d[:])
            make_identity(nc, i32[:])
            nc.gpsimd.memset(ones[:], 1.0)

            # transposed input for the dense phase: xT[p, k, r] = x[r, 128k+p]
            for k in range(NK):
                nc.sync.dma_start_transpose(
                    xT_sb[:, k, :], xin_d[:, k * 128 : (k + 1) * 128]
                )

            # initial state
            hT_cur = hTp.tile([128, NK, BH], BF16, tag="hT")
            c_cur = cp.tile([BH, H], F32, tag="c")
            nc.sync.dma_start(hT_cur[:], hT0_d[:])
            nc.sync.dma_start(c_cur[:], c0_d[:])

            # ---- phase 1: dense x-part  Xg = x @ W_ih.T + bias ----
            for n in range(NCH):
                nsl = slice(n * 512, (n + 1) * 512)
                wihn = wpool.tile([128, NK, 512], BF16)
                nc.sync.dma_start(wihn[:], wih_d[:, :, nsl])
                for r in range(R // 128):
                    pd = psd.tile([128, 512], F32)
                    nc.tensor.matmul(
                        pd[:], ones[:], brow_sb[:, nsl], start=True, stop=False
                    )
                    for k in range(NK):
                        nc.tensor.matmul(
                            pd[:],
                            xT_sb[:, k, r * 128 : (r + 1) * 128],
                            wihn[:, k, :],
                            start=False,
                            stop=(k == NK - 1),
                        )
                    ev = xev.tile([128, 512], BF16)
                    nc.vector.tensor_copy(ev[:], pd[:])
                    nc.sync.dma_start(Xg_d[r * 128 : (r + 1) * 128, nsl], ev[:])

            # ---- phase 2: recurrent scan ----
            for t in range(C):
                xt = xtp.tile([BH, G], BF16)
                nc.sync.dma_start(xt[:], Xg_d[t * BH : (t + 1) * BH, :])

                gall = gp.tile([BH, G], BF16)
                for n in range(NCH):
                    nsl = slice(n * 512, (n + 1) * 512)
                    pg = psg.tile([64, 512], F32)
                    # group A (psum rows 0:32): x-part inject + K tiles 0..3
                    nc.tensor.matmul(
                        pg[0:BH, :], i32[:], xt[:, nsl], start=True, stop=False
                    )
                    for k in range(4):
                        nc.tensor.matmul(
                            pg[0:BH, :],
                            hT_cur[:, k, :],
                            whh_sb[:, k, nsl],
                            start=False,
                            stop=(k == 3),
                        )
                    # group B (psum rows 32:64): K tiles 4..7 (concurrent col group)
                    for k in range(4, NK):
                        nc.tensor.matmul(
                            pg[BH : 2 * BH, :],
                            hT_cur[:, k, :],
                            whh_sb[:, k, nsl],
                            start=(k == 4),
                            stop=(k == NK - 1),
                        )
                    # fold: gall = A + B
                    nc.scalar.copy(gall[:, nsl], pg[0:BH, :])
                    nc.vector.tensor_add(gall[:, nsl], gall[:, nsl], pg[BH : 2 * BH, :])

                # gate order in gall: [i | f | o | g]
                sig = ep.tile([BH, 3 * H], BF16, tag="sig")
                nc.scalar.activation(
                    sig[:], gall[:, 0 : 3 * H], mybir.ActivationFunctionType.Sigmoid
                )
                tg = ep.tile([BH, H], BF16, tag="tg")
                nc.scalar.activation(
                    tg[:], gall[:, 3 * H : G], mybir.ActivationFunctionType.Tanh
                )
                t1 = ep.tile([BH, H], BF16, tag="t1")
                nc.vector.tensor_mul(t1[:], sig[:, 0:H], tg[:])
                t2 = ep.tile([BH, H], F32, tag="t2")
                nc.vector.tensor_mul(t2[:], sig[:, H : 2 * H], c_cur[:])
                c_new = cp.tile([BH, H], F32, tag="c")
                nc.vector.tensor_add(c_new[:], t2[:], t1[:])
                tc2 = ep.tile([BH, H], BF16, tag="tc2")
                nc.scalar.activation(
                    tc2[:], c_new[:], mybir.ActivationFunctionType.Tanh
                )
                h_sb = hp.tile([BH, H], BF16)
                nc.vector.tensor_mul(h_sb[:], sig[:, 2 * H : 3 * H], tc2[:])

                nc.sync.dma_start(hseq_d[t], h_sb[:])

                # transpose h -> hT for the next step's stationary operand
                pT = pst.tile([128, NK, BH], BF16)
                for k in range(NK):
                    nc.tensor.transpose(
                        pT[:, k, :], h_sb[:, k * 128 : (k + 1) * 128], i32[:]
                    )
                hT_new = hTp.tile([128, NK, BH], BF16, tag="hT")
                nc.vector.tensor_copy(hT_new[:], pT[:])

                hT_cur = hT_new
                c_cur = c_new

            nc.sync.dma_start(hTn_d[:], hT_cur[:])
            nc.sync.dma_start(cn_d[:], c_cur[:])

    nc.compile()
    return nc


# ---------------- host side ----------------

_PERM = np.concatenate(
    [np.arange(0, H), np.arange(H, 2 * H), np.arange(3 * H, 4 * H),
     np.arange(2 * H, 3 * H)]
)  # torch gate order i,f,g,o -> kernel order i,f,o,g


def _stage_w(w):
    """[G, I] fp32 (already gate-permuted) -> [128, NK, G] bf16 K-tile layout."""
    wt = np.ascontiguousarray(w.T)                     # [I, G]
    return np.ascontiguousarray(
        wt.reshape(NK, 128, G).transpose(1, 0, 2)
    ).astype(NPBF)


def _stage_hT(h):
    """[BH, H] fp32 -> [128, NK, BH] bf16 transposed K-tile layout."""
    return np.ascontiguousarray(
        h.T.reshape(NK, 128, BH).transpose(1, 0, 2)
    ).astype(NPBF)


def _hT_identity_check():
    pass


def kernel(**inputs):
    inputs = {k: np.asarray(v) for k, v in inputs.items()}
    x = inputs["input"]                    # [B, T, I] fp32
    enc_h = inputs["encoder_h"]            # [B, 2H]
    enc_c = inputs["encoder_c"]

    if "nc" not in _CACHE:
        _CACHE["nc"] = _build_nc()
    nc = _CACHE["nc"]

    # per-cell weight staging
    cells = ["0f", "0b", "1f", "1b"]
    wih_cell, whh_cell, brow_cell = {}, {}, {}
    for s in cells:
        wih = inputs[f"W_ih_{s}"]
        if s == "0b":
            wih = wih[:, ::-1]             # feature-reversed input
        wih_cell[s] = _stage_w(wih[_PERM])
        whh_cell[s] = _stage_w(inputs[f"W_hh_{s}"][_PERM])
        brow_cell[s] = (
            (inputs[f"b_ih_{s}"] + inputs[f"b_hh_{s}"])[_PERM]
            .reshape(1, G).astype(NPBF)
        )

    hf, hb = enc_h[:, :H], enc_h[:, H:]
    cf, cb = enc_c[:, :H], enc_c[:, H:]
    h_init = {"0f": hf, "0b": hb, "1f": hf, "1b": hb}
    c_init = {"0f": cf, "0b": cb, "1f": cf, "1b": cb}

    # core -> (cell, half). layer0 on cores 0..3, layer1 on cores 4..7
    core_map = [("0f", 0), ("0f", 1), ("0b", 0), ("0b", 1),
                ("1f", 0), ("1f", 1), ("1b", 0), ("1b", 1)]

    # x chunks in (t, b)-major row layout, bf16
    xbf = x.astype(NPBF)
    x_chunk = {}
    for half in (0, 1):
        xs = xbf[half * BH : (half + 1) * BH]          # [BH, T, I]
        for j in range(T // C):
            ch = np.ascontiguousarray(
                xs[:, j * C : (j + 1) * C].transpose(1, 0, 2).reshape(R, I)
            )
            x_chunk[(half, j)] = ch

    state = {}
    for core, (cell, half) in enumerate(core_map):
        state[core] = {
            "hT": _stage_hT(h_init[cell][half * BH : (half + 1) * BH]),
            "c": np.ascontiguousarray(
                c_init[cell][half * BH : (half + 1) * BH]
            ).astype(np.float32),
        }

    kernel.launch_s = []
    zeros_x = np.zeros((R, I), NPBF)
    nchunks = T // C
    h0_chunks = {}                         # (layer0 core, chunk j) -> hseq bf16
    out = np.zeros((B, T, 2 * H), np.float32)
    exec_ns = []

    for k in range(nchunks + 1):
        in_maps = []
        active = []
        for core, (cell, half) in enumerate(core_map):
            layer = 0 if cell in ("0f", "0b") else 1
            j = k if layer == 0 else k - 1
            if 0 <= j < nchunks:
                if layer == 0:
                    xin = x_chunk[(half, j)]
                else:
                    xin = h0_chunks[(core - 4, j)]
                active.append((core, cell, half, layer, j))
            else:
                xin = zeros_x
            in_maps.append({
                "xin": xin,
                "wihT": wih_cell[cell],
                "whhT": whh_cell[cell],
                "brow": brow_cell[cell],
                "hT0": state[core]["hT"],
                "c0": state[core]["c"],
            })

        import time as _time
        _t0 = _time.time()
        res = run_bass_kernel_spmd(
            nc, in_maps, core_ids=list(range(8)),
            trace=bool(os.environ.get("BASS_LSTM_PROFILE")),
        )
        kernel.launch_s.append(_time.time() - _t0)
        if res.exec_time_ns is not None:
            exec_ns.append(res.exec_time_ns)
        kernel.last_res = res

        for core, cell, half, layer, j in active:
            r = res.results[core]
            state[core]["hT"] = np.asarray(r["hTn"])
            state[core]["c"] = np.asarray(r["cn"])
            hseq = np.asarray(r["hseq"])   # [C, BH, H] bf16
            if layer == 0:
                h0_chunks[(core, j)] = np.ascontiguousarray(
                    hseq.reshape(R, I)
                )
            else:
                fwd = cell == "1f"
                bsl = slice(half * BH, (half + 1) * BH)
                gsl = slice(0, H) if fwd else slice(H, 2 * H)
                out[bsl, j * C : (j + 1) * C, gsl] = (
                    hseq.transpose(1, 0, 2).astype(np.float32)
                )

    h_fin = np.ascontiguousarray(out[:, -1, :])
    c_fin = np.zeros((B, 2 * H), np.float32)
    for core, (cell, half) in enumerate(core_map):
        if cell == "1f":
            c_fin[half * BH : (half + 1) * BH, 0:H] = state[core]["c"]
        elif cell == "1b":
            c_fin[half * BH : (half + 1) * BH, H : 2 * H] = state[core]["c"]

    kernel.last_exec_ns = exec_ns
    return out, h_fin, c_fin


# revision 12
# speedup vs baseline: 1.4458x; 1.4458x over previous
"""Bidirectional 2-layer LSTM decoder on 8 Trainium2 NeuronCores.

Structure (hardcoded for B=64, T=256, I=H=1024):
  - reference semantics: 4 LSTM cells per step (layer0 fwd/bwd on x /
    feature-reversed x, layer1 fwd/bwd on layer0 outputs), all scanning
    forward in time.
  - sharding: core = (cell, batch-half) -> 8 cores, 32 batch rows each.
    Zero communication inside a launch; layer-1 cores consume layer-0 h
    sequences handed off host-side between chunked launches (software
    pipeline with chunk size C steps).
  - per-core kernel (one chunk): dense GEMM for the x-part of the gates
    (x @ W_ih.T + bias), interleaved slab-wise with C recurrent steps.
    Recurrent step: h @ W_hh.T in two concurrent PE column-group chains
    (gate chunks 0-3 on cols 0-31, chunks 4-7 on cols 32-63, two chunks
    per PSUM bank, full-K accumulation, no partial-sum folds), ScalarE
    activations read PSUM directly, VectorE does the cell-state math,
    PE transposes h for the next step's stationary operand.
  - all matmuls bf16 (fp32 is 4x slower on the PE); cell state bf16
    (validated ~1e-2 max rel err vs fp32 reference).
  - gate order within the 4096 gate rows: [i | g | f | o] so that
    i/f arrive in early rounds and the c/h math overlaps later rounds.
"""

import os
import sys

sys.path.insert(0, "/opt/trn_rl_repo")

import numpy as np
import ml_dtypes

import concourse.bass as bass
import concourse.mybir as mybir
import concourse.tile as tile
from concourse import bacc
from concourse.bass_utils import run_bass_kernel_spmd
from concourse.masks import make_identity


def _install_ntff_shim():
    """Provide antenv.axon_hooks (missing in this image) so that
    run_bass_kernel_spmd(trace=True) can capture NTFF profiles."""
    import types

    try:
        import antenv.axon_hooks  # noqa: F401
        return
    except ImportError:
        pass
    try:
        import antenv
        from trn_agent_boot.trn_boot import _ntff_profile_via_ctypes

        mod = types.ModuleType("antenv.axon_hooks")
        _h = [None]
        mod.set_axon_ntff_profile_hook = lambda h: _h.__setitem__(0, h)
        mod.get_axon_ntff_profile_hook = lambda: _h[0]
        sys.modules["antenv.axon_hooks"] = mod
        antenv.axon_hooks = mod
        hook = _ntff_profile_via_ctypes("/opt/axon/libaxon_pjrt.so")
        mod.set_axon_ntff_profile_hook(hook)

        import concourse.bass_utils as _bu
        _bu.upload_artifacts = lambda tmpdir: tmpdir  # no bucket in container
    except Exception:
        pass


_install_ntff_shim()

BF16 = mybir.dt.bfloat16
F32 = mybir.dt.float32
NPBF = ml_dtypes.bfloat16
ACTF = mybir.ActivationFunctionType

B, T, I, H = 64, 256, 1024, 1024
G = 4 * H            # gate width
NK = H // 128        # K tiles (8)
BH = B // 2          # batch half = 32
C = 64               # scan steps per launch
R = C * BH           # dense rows per chunk

_CACHE = {}


def _build_nc(C=C, R=None, debug=False):
    R = C * BH if R is None else R
    nslabs = R // 128
    nc = bacc.Bacc("TRN2", target_bir_lowering=False, debug=debug)

    xin_d = nc.dram_tensor("xin", (R, I), BF16, kind="ExternalInput")
    wih_d = nc.dram_tensor("wihT", (128, NK, G), BF16, kind="ExternalInput")
    whh_d = nc.dram_tensor("whhT", (128, NK, G), BF16, kind="ExternalInput")
    brow_d = nc.dram_tensor("brow", (1, G), BF16, kind="ExternalInput")
    hT0_d = nc.dram_tensor("hT0", (128, NK, BH), BF16, kind="ExternalInput")
    c0_d = nc.dram_tensor("c0", (BH, H), BF16, kind="ExternalInput")

    hseq_d = nc.dram_tensor("hseq", (C, BH, H), BF16, kind="ExternalOutput")
    hTn_d = nc.dram_tensor("hTn", (128, NK, BH), BF16, kind="ExternalOutput")
    cn_d = nc.dram_tensor("cn", (BH, H), BF16, kind="ExternalOutput")

    Xg_d = nc.dram_tensor("Xg", (R, G), BF16, kind="Internal")

    with tile.TileContext(nc) as tc:
        with (
            tc.tile_pool(name="const", bufs=1) as cst,
            tc.tile_pool(name="xslab", bufs=3) as xslab,
            tc.tile_pool(name="xtp", bufs=3) as xtp,
            tc.tile_pool(name="ep", bufs=2) as ep,
            tc.tile_pool(name="cp", bufs=2) as cp,
            tc.tile_pool(name="hp", bufs=2) as hp,
            tc.tile_pool(name="hTp", bufs=2) as hTp,
            tc.tile_pool(name="xev", bufs=4) as xev,
            tc.tile_pool(name="psd", bufs=2, space="PSUM") as psd,
            tc.tile_pool(name="psg", bufs=4, space="PSUM") as psg,
            tc.tile_pool(name="pst", bufs=2, space="PSUM") as pst,
        ):
            # ---- constants / weights ----
            wih_sb = cst.tile([128, NK, G], BF16)
            whh_sb = cst.tile([128, NK, G], BF16)
            brow_sb = cst.tile([1, G], BF16)
            i32 = cst.tile([32, 32], BF16)
            ones = cst.tile([1, 128], BF16)

            nc.sync.dma_start(wih_sb[:], wih_d[:])
            nc.sync.dma_start(whh_sb[:], whh_d[:])
            nc.sync.dma_start(brow_sb[:], brow_d[:])
            make_identity(nc, i32[:])
            nc.gpsimd.memset(ones[:], 1.0)

            hT_cur = hTp.tile([128, NK, BH], BF16, tag="hT")
            c_cur = cp.tile([BH, H], BF16, tag="c")
            nc.sync.dma_start(hT_cur[:], hT0_d[:])
            nc.sync.dma_start(c_cur[:], c0_d[:])

            # ---- dense slab: 128 rows of Xg = x @ W_ih.T + bias ----
            def emit_dense_slab(r):
                rsl = slice(r * 128, (r + 1) * 128)
                xTs = xslab.tile([128, NK, 128], BF16)
                for k in range(NK):
                    nc.sync.dma_start_transpose(
                        xTs[:, k, :], xin_d[rsl, k * 128 : (k + 1) * 128]
                    )
                for n in range(8):
                    nsl = slice(n * 512, (n + 1) * 512)
                    pd = psd.tile([128, 512], F32)
                    nc.tensor.matmul(
                        pd[:], ones[:], brow_sb[:, nsl], start=True, stop=False
                    )
                    for k in range(NK):
                        nc.tensor.matmul(
                            pd[:], xTs[:, k, :], wih_sb[:, k, nsl],
                            start=False, stop=(k == NK - 1),
                        )
                    ev = xev.tile([128, 512], BF16)
                    if n % 2 == 0:
                        nc.vector.tensor_copy(ev[:], pd[:])
                    else:
                        nc.scalar.copy(ev[:], pd[:])
                    nc.sync.dma_start(Xg_d[rsl, nsl], ev[:])

            for _s in range(min(2, nslabs)):
                emit_dense_slab(_s)
            next_slab = min(2, nslabs)

            # gate chunk map (columns of the staged 4096-wide gate space):
            #   chunks 0-1 = i, 2-3 = g  (low rows 0:32 of psum banks 0-3)
            #   chunks 4-5 = f, 6-7 = o  (high rows 32:64, concurrent col grp)
            # round p accumulates chunks (p, p+4) into one psum bank.
            for t in range(C):
                xt = xtp.tile([BH, G], BF16)
                nc.sync.dma_start(xt[:], Xg_d[t * BH : (t + 1) * BH, :])

                si = ep.tile([BH, H], BF16, tag="si")
                sf = ep.tile([BH, H], BF16, tag="sf")
                so = ep.tile([BH, H], BF16, tag="so")
                tg = ep.tile([BH, H], BF16, tag="tg")
                t2 = ep.tile([BH, H], BF16, tag="t2")
                tc2 = ep.tile([BH, H], BF16, tag="tc2")
                c_new = cp.tile([BH, H], BF16, tag="c")
                h_sb = hp.tile([BH, H], BF16)
                pT = pst.tile([128, NK, BH], BF16)

                pgs = []
                for p in range(4):
                    lo = slice(p * 512, (p + 1) * 512)
                    hi = slice((p + 4) * 512, (p + 5) * 512)
                    pg = psg.tile([64, 512], F32)
                    pgs.append(pg)
                    for k in range(NK):
                        nc.tensor.matmul(
                            pg[0:BH, :], hT_cur[:, k, :], whh_sb[:, k, lo],
                            start=(k == 0), stop=False, skip_group_check=True,
                        )
                        nc.tensor.matmul(
                            pg[BH:64, :], hT_cur[:, k, :], whh_sb[:, k, hi],
                            start=(k == 0), stop=False, skip_group_check=True,
                        )
                    nc.tensor.matmul(
                        pg[0:BH, :], i32[:], xt[:, lo], start=False, stop=True,
                        skip_group_check=True,
                    )
                    nc.tensor.matmul(
                        pg[BH:64, :], i32[:], xt[:, hi], start=False, stop=True,
                        skip_group_check=True,
                    )

                    hsl = slice((p % 2) * 512, (p % 2) * 512 + 512)
                    if p < 2:
                        nc.scalar.activation(si[:, hsl], pg[0:BH, :], ACTF.Sigmoid)
                        nc.scalar.activation(sf[:, hsl], pg[BH:64, :], ACTF.Sigmoid)
                        if p == 1:
                            nc.vector.tensor_mul(t2[:], sf[:], c_cur[:])
                    else:
                        nc.scalar.activation(tg[:, hsl], pg[0:BH, :], ACTF.Tanh)
                        nc.scalar.activation(so[:, hsl], pg[BH:64, :], ACTF.Sigmoid)
                        # half of the cell-state math as soon as its gates land
                        nc.vector.tensor_mul(si[:, hsl], si[:, hsl], tg[:, hsl])
                        nc.vector.tensor_add(c_new[:, hsl], t2[:, hsl], si[:, hsl])
                        nc.scalar.activation(tc2[:, hsl], c_new[:, hsl], ACTF.Tanh)
                        nc.vector.tensor_mul(h_sb[:, hsl], so[:, hsl], tc2[:, hsl])
                        for k in range(4 * (p - 2), 4 * (p - 1)):
                            nc.tensor.transpose(
                                pT[:, k, :], h_sb[:, k * 128 : (k + 1) * 128],
                                i32[:],
                            )

                nc.sync.dma_start(hseq_d[t], h_sb[:])
                hT_new = hTp.tile([128, NK, BH], BF16, tag="hT")
                nc.vector.tensor_copy(hT_new[:], pT[:])
                hT_cur = hT_new
                c_cur = c_new

                if t % 4 == 2 and next_slab < nslabs:
                    emit_dense_slab(next_slab)
                    next_slab += 1

            while next_slab < nslabs:
                emit_dense_slab(next_slab)
                next_slab += 1

            nc.sync.dma_start(hTn_d[:], hT_cur[:])
            nc.sync.dma_start(cn_d[:], c_cur[:])

    nc.compile()
    return nc


# ---------------- host side ----------------

# torch gate-row order [i f g o] -> kernel order [i g f o]
_PERM = np.concatenate(
    [np.arange(0, H), np.arange(2 * H, 3 * H), np.arange(H, 2 * H),
     np.arange(3 * H, 4 * H)]
)


def _stage_w(w):
    """[G, I] fp32 (already gate-permuted) -> [128, NK, G] bf16 K-tile layout."""
    wt = np.ascontiguousarray(w.T)                     # [I, G]
    return np.ascontiguousarray(
        wt.reshape(NK, 128, G).transpose(1, 0, 2)
    ).astype(NPBF)


def _stage_hT(h):
    """[BH, H] fp32 -> [128, NK, BH] bf16 transposed K-tile layout."""
    return np.ascontiguousarray(
        h.T.reshape(NK, 128, BH).transpose(1, 0, 2)
    ).astype(NPBF)


def kernel(**inputs):
    inputs = {k: np.asarray(v) for k, v in inputs.items()}
    x = inputs["input"]                    # [B, T, I] fp32
    enc_h = inputs["encoder_h"]
    enc_c = inputs["encoder_c"]

    if "nc" not in _CACHE:
        _CACHE["nc"] = _build_nc()
    nc = _CACHE["nc"]

    cells = ["0f", "0b", "1f", "1b"]
    wih_cell, whh_cell, brow_cell = {}, {}, {}
    for s in cells:
        wih = inputs[f"W_ih_{s}"]
        if s == "0b":
            wih = wih[:, ::-1]             # feature-reversed input
        wih_cell[s] = _stage_w(wih[_PERM])
        whh_cell[s] = _stage_w(inputs[f"W_hh_{s}"][_PERM])
        brow_cell[s] = (
            (inputs[f"b_ih_{s}"] + inputs[f"b_hh_{s}"])[_PERM]
            .reshape(1, G).astype(NPBF)
        )

    hf, hb = enc_h[:, :H], enc_h[:, H:]
    cf, cb = enc_c[:, :H], enc_c[:, H:]
    h_init = {"0f": hf, "0b": hb, "1f": hf, "1b": hb}
    c_init = {"0f": cf, "0b": cb, "1f": cf, "1b": cb}

    core_map = [("0f", 0), ("0f", 1), ("0b", 0), ("0b", 1),
                ("1f", 0), ("1f", 1), ("1b", 0), ("1b", 1)]

    xbf = x.astype(NPBF)
    x_chunk = {}
    for half in (0, 1):
        xs = xbf[half * BH : (half + 1) * BH]          # [BH, T, I]
        for j in range(T // C):
            x_chunk[(half, j)] = np.ascontiguousarray(
                xs[:, j * C : (j + 1) * C].transpose(1, 0, 2).reshape(R, I)
            )

    state = {}
    for core, (cell, half) in enumerate(core_map):
        state[core] = {
            "hT": _stage_hT(h_init[cell][half * BH : (half + 1) * BH]),
            "c": np.ascontiguousarray(
                c_init[cell][half * BH : (half + 1) * BH]
            ).astype(NPBF),
        }

    kernel.launch_s = []
    zeros_x = np.zeros((R, I), NPBF)
    nchunks = T // C
    h0_chunks = {}
    out = np.zeros((B, T, 2 * H), np.float32)
    exec_ns = []

    for k in range(nchunks + 1):
        in_maps = []
        active = []
        for core, (cell, half) in enumerate(core_map):
            layer = 0 if cell in ("0f", "0b") else 1
            j = k if layer == 0 else k - 1
            if 0 <= j < nchunks:
                xin = x_chunk[(half, j)] if layer == 0 else h0_chunks[(core - 4, j)]
                active.append((core, cell, half, layer, j))
            else:
                xin = zeros_x
            in_maps.append({
                "xin": xin,
                "wihT": wih_cell[cell],
                "whhT": whh_cell[cell],
                "brow": brow_cell[cell],
                "hT0": state[core]["hT"],
                "c0": state[core]["c"],
            })

        import time as _time
        _t0 = _time.time()
        res = run_bass_kernel_spmd(
            nc, in_maps, core_ids=list(range(8)),
            trace=bool(os.environ.get("BASS_LSTM_PROFILE")),
        )
        kernel.launch_s.append(_time.time() - _t0)
        if res.exec_time_ns is not None:
            exec_ns.append(res.exec_time_ns)
        kernel.last_res = res

        for core, cell, half, layer, j in active:
            r = res.results[core]
            state[core]["hT"] = np.asarray(r["hTn"])
            state[core]["c"] = np.asarray(r["cn"])
            hseq = np.asarray(r["hseq"])   # [C, BH, H] bf16
            if layer == 0:
                h0_chunks[(core, j)] = np.ascontiguousarray(hseq.reshape(R, I))
            else:
                bsl = slice(half * BH, (half + 1) * BH)
                gsl = slice(0, H) if cell == "1f" else slice(H, 2 * H)
                out[bsl, j * C : (j + 1) * C, gsl] = (
                    hseq.transpose(1, 0, 2).astype(np.float32)
                )

    h_fin = np.ascontiguousarray(out[:, -1, :])
    c_fin = np.zeros((B, 2 * H), np.float32)
    for core, (cell, half) in enumerate(core_map):
        if cell == "1f":
            c_fin[half * BH : (half + 1) * BH, 0:H] = state[core]["c"].astype(
                np.float32
            )
        elif cell == "1b":
            c_fin[half * BH : (half + 1) * BH, H : 2 * H] = state[core][
                "c"
            ].astype(np.float32)

    kernel.last_exec_ns = exec_ns
    return out, h_fin, c_fin
